# revision 18
# baseline (speedup 1.0000x reference)
"""GQA causal-attention prefill kernel for 8 TRN2 NeuronCores.

Sharding: tensor-parallel over heads. Core c owns q-heads {2c, 2c+1} and
kv-head c (whole GQA group). Each core computes its heads' attention and a
partial output projection; the host sums the 8 partials (no collectives).

Layout: activations kept transposed [feature, token] on-chip so every matmul
uses natural operand layouts. Matmuls run in bf16 (PSUM accumulation in f32).
Softmax skips max-subtraction: scores are bounded by ||q||*||k||/sqrt(D) ~ 12,
safe in f32/bf16 exp. The softmax denominator is accumulated on the Vector and
GpSimd engines (block-wise adds) with a single ones-matmul per chunk for the
partition reduction, keeping the PE on QK/AV work. rotate_half is a
partition-swapping SBUF->SBUF DMA with the sign folded into the sin table.
"""
import os
import sys

for _p in ("/opt/trn_rl_repo", "/root/.axon_site/_ro/trn_rl_repo"):
    if os.path.isdir(_p) and _p not in sys.path:
        sys.path.append(_p)

import numpy as np
import ml_dtypes
import concourse.bacc as bacc
import concourse.bass_isa as bass_isa
import concourse.mybir as mybir
import concourse.tile as tile
from concourse.bass_utils import run_bass_kernel_spmd

B, S, DIM = 2, 2048, 2048
H, KVH, D = 16, 8, 128
EPS = 1e-6
NCORES = 8
HL = H // NCORES            # q heads per core
SQC = 512                   # sequence chunk (matmul moving dim)
NJ = S // SQC               # chunks per batch
KT = DIM // 128             # contraction tiles for the projections
BS = B * S
SCALE = 1.0 / float(np.sqrt(D))

F32 = mybir.dt.float32
BF16 = mybir.dt.bfloat16
AF = mybir.ActivationFunctionType


def build(debug_dumps=False):
    nc = bacc.Bacc("TRN2", target_bir_lowering=False, debug=False,
                   num_devices=NCORES)
    xt = nc.dram_tensor("xt", [DIM, BS], BF16, kind="ExternalInput").ap()
    wq = nc.dram_tensor("wq", [128, KT * HL * D], BF16, kind="ExternalInput").ap()
    wk = nc.dram_tensor("wk", [128, KT * D], BF16, kind="ExternalInput").ap()
    wv = nc.dram_tensor("wv", [128, KT * D], BF16, kind="ExternalInput").ap()
    wo = nc.dram_tensor("wo", [HL * D, DIM], BF16, kind="ExternalInput").ap()
    gq = nc.dram_tensor("gq", [D, 1], F32, kind="ExternalInput").ap()
    gk = nc.dram_tensor("gk", [D, 1], F32, kind="ExternalInput").ap()
    cost = nc.dram_tensor("cost", [128, S], BF16, kind="ExternalInput").ap()
    sint = nc.dram_tensor("sint", [128, S], BF16, kind="ExternalInput").ap()
    msk = nc.dram_tensor("msk", [128, 128], BF16, kind="ExternalInput").ap()
    on128 = nc.dram_tensor("on128", [128, 128], BF16, kind="ExternalInput").ap()
    giq = nc.dram_tensor("giq", [128, 128], BF16, kind="ExternalInput").ap()
    gik = nc.dram_tensor("gik", [128, 128], BF16, kind="ExternalInput").ap()
    idn = nc.dram_tensor("idn", [128, 128], BF16, kind="ExternalInput").ap()
    out = nc.dram_tensor("out", [DIM, BS], BF16, kind="ExternalOutput").ap()
    dbg = {}
    if debug_dumps:
        for nm in ("qt0d", "ktd", "vbd", "ao0d"):
            dbg[nm] = nc.dram_tensor(nm, [128, S], BF16,
                                     kind="ExternalOutput").ap()

    with tile.TileContext(nc) as tc:
        with tc.tile_pool(name="const", bufs=1) as cp, \
             tc.tile_pool(name="xp", bufs=44) as xp, \
             tc.tile_pool(name="persist", bufs=2) as pp, \
             tc.tile_pool(name="wrk", bufs=5) as wrk, \
             tc.tile_pool(name="pairs", bufs=4) as prp, \
             tc.tile_pool(name="ep", bufs=12) as epool, \
             tc.tile_pool(name="oop", bufs=12) as oop, \
             tc.tile_pool(name="ps_big", bufs=5, space="PSUM") as psb, \
             tc.tile_pool(name="ps_sm", bufs=1, space="PSUM") as pssm, \
             tc.tile_pool(name="ps_av", bufs=1, space="PSUM") as psa, \
             tc.tile_pool(name="ps_row", bufs=1, space="PSUM") as psr:

            # ---- constants / weights in SBUF ----
            # startup DMAs spread across 5 queues so the first projection
            # matmuls (wq q0 + wk q0 + wv q0 + x chunk0) are fed ASAP
            wq_sb = cp.tile([128, KT * HL * D], BF16)
            wk_sb = cp.tile([128, KT * D], BF16)
            wv_sb = cp.tile([128, KT * D], BF16)
            on128_sb = cp.tile([128, 128], BF16)
            giq_sb = cp.tile([128, 128], BF16)
            gik_sb = cp.tile([128, 128], BF16)
            idn_sb = cp.tile([128, 128], BF16)
            gq_sb = cp.tile([D, 1], F32)
            gk_sb = cp.tile([D, 1], F32)
            cos_sb = cp.tile([128, S], BF16)
            sin_sb = cp.tile([128, S], BF16)
            msk_sb = cp.tile([128, 128], BF16)
            wo_sb = [cp.tile([128, DIM], BF16, name=f"wo{h}") for h in range(HL)]

            qw4 = KT * HL * D // 4
            kw4 = KT * D // 4
            # gpsimd queue: quarter 0 of each weight first (the kb=0 matmul
            # group needs them), small attention constants, the rest, then wo
            nc.gpsimd.dma_start(out=wq_sb[:, 0:qw4], in_=wq[:, 0:qw4])
            nc.gpsimd.dma_start(out=wk_sb[:, 0:kw4], in_=wk[:, 0:kw4])
            nc.gpsimd.dma_start(out=wv_sb[:, 0:kw4], in_=wv[:, 0:kw4])
            nc.gpsimd.dma_start(out=on128_sb[:], in_=on128)
            nc.gpsimd.dma_start(out=giq_sb[:], in_=giq)
            nc.gpsimd.dma_start(out=gik_sb[:], in_=gik)
            nc.gpsimd.dma_start(out=idn_sb[:], in_=idn)
            for q4 in range(1, 4):
                nc.gpsimd.dma_start(out=wq_sb[:, q4 * qw4:(q4 + 1) * qw4],
                                    in_=wq[:, q4 * qw4:(q4 + 1) * qw4])
                nc.gpsimd.dma_start(out=wk_sb[:, q4 * kw4:(q4 + 1) * kw4],
                                    in_=wk[:, q4 * kw4:(q4 + 1) * kw4])
                nc.gpsimd.dma_start(out=wv_sb[:, q4 * kw4:(q4 + 1) * kw4],
                                    in_=wv[:, q4 * kw4:(q4 + 1) * kw4])
            nc.gpsimd.dma_start(out=cos_sb[:], in_=cost)
            nc.gpsimd.dma_start(out=sin_sb[:], in_=sint)
            nc.gpsimd.dma_start(out=msk_sb[:], in_=msk)
            # sync queue: gammas up front (needed by the first rmsnorm);
            # cos/sin are interleaved into the sc==0 x-tile stream below
            nc.sync.dma_start(out=gq_sb[:], in_=gq)
            nc.sync.dma_start(out=gk_sb[:], in_=gk)

            for b in range(B):
                base = b * S
                fence_ref = [None]
                # per-batch persistent buffers (double-buffered across b)
                qt_buf = [pp.tile([128, S], BF16, tag=f"qt{h}", name=f"qt{h}")
                          for h in range(HL)]
                kt_buf = pp.tile([128, S], BF16, tag="ktb")
                v_buf = pp.tile([128, S], BF16, tag="vb")
                ao_buf = [pp.tile([128, S], BF16, tag=f"ao{h}", name=f"ao{h}")
                          for h in range(HL)]
                # ======== phase P: projections + norm + rope ========
                for sc in range(NJ):
                    col = sc * SQC
                    xk = []
                    first = (b == 0 and sc == 0)
                    for kt in range(KT):
                        t = xp.tile([128, SQC], BF16, tag="x", name=f"x{kt}")
                        eng = nc.scalar if (first and kt % 2) else nc.sync
                        eng.dma_start(
                            out=t[:],
                            in_=xt[kt * 128:(kt + 1) * 128,
                                   base + col:base + col + SQC])
                        xk.append(t)
                    qp = [psb.tile([128, SQC], F32, tag="big", name=f"qp{h}")
                          for h in range(HL)]
                    kp = psb.tile([128, SQC], F32, tag="big")
                    vp = psa.tile([128, SQC], F32, tag="av", name="vp")
                    for kb in range(0, KT, 4):
                        for ci, (pt, base_col, stride) in enumerate(
                                [(qp[0], 0, HL * D), (qp[1], D, HL * D),
                                 (kp, 0, D), (vp, 0, D)]):
                            wsb = [wq_sb, wq_sb, wk_sb, wv_sb][ci]
                            for kt in range(kb, kb + 4):
                                st, sp = kt == 0, kt == KT - 1
                                nc.tensor.matmul(
                                    pt[:],
                                    wsb[:, kt * stride + base_col:
                                        kt * stride + base_col + D],
                                    xk[kt][:], start=st, stop=sp)

                    # -- rmsnorm + gamma + rope for q heads and k --
                    for ps, g_col, gi_sb, dest in (
                            [(qp[h], gq_sb, giq_sb, qt_buf[h])
                             for h in range(HL)]
                            + [(kp, gk_sb, gik_sb, kt_buf)]):
                        tsb = wrk.tile([128, SQC], BF16, tag="tsb")
                        nc.vector.tensor_scalar_mul(tsb[:], ps[:], g_col[:])
                        # ||q||^2 via (gamma*q)^2 summed against 1/gamma^2
                        # weights: keeps Square off the scalar engine (its
                        # activation table would thrash against Exp)
                        sqr = wrk.tile([128, SQC], BF16, tag="sqr")
                        nc.vector.tensor_mul(sqr[:], tsb[:], tsb[:])
                        ssum = psr.tile([128, SQC], F32, tag="row")
                        nc.tensor.matmul(ssum[:], gi_sb[:], sqr[:],
                                         start=True, stop=True)
                        sdn = wrk.tile([128, SQC], F32, tag="sdn")
                        nc.scalar.activation(sdn[:], ssum[:], AF.Sqrt,
                                             scale=1.0 / D)
                        bcs = wrk.tile([128, SQC], F32, tag="bcs")
                        nc.vector.reciprocal_approx_fast(bcs[:], sdn[:])
                        # rotate_half: partition-swap via SBUF->SBUF DMA;
                        # the sign lives in the sin table (rows 0-63 are -sin)
                        rots = wrk.tile([128, SQC], BF16, tag="rots")
                        nc.gpsimd.dma_start(out=rots[0:64, :],
                                            in_=tsb[64:128, :])
                        nc.gpsimd.dma_start(out=rots[64:128, :],
                                            in_=tsb[0:64, :])
                        c_sl = cos_sb[:, col:col + SQC]
                        s_sl = sin_sb[:, col:col + SQC]
                        m1 = wrk.tile([128, SQC], BF16, tag="m1")
                        m2 = wrk.tile([128, SQC], BF16, tag="m2")
                        nc.vector.tensor_mul(m1[:], tsb[:], c_sl)
                        nc.vector.tensor_mul(m2[:], rots[:], s_sl)
                        u = wrk.tile([128, SQC], BF16, tag="m1")
                        nc.vector.tensor_add(u[:], m1[:], m2[:])
                        nc.vector.tensor_mul(dest[:, col:col + SQC],
                                             u[:], bcs[:])

                    # -- V: copy + transpose to natural [s, d] layout --
                    vt = wrk.tile([128, SQC], BF16, tag="tsb")
                    nc.scalar.activation(vt[:], vp[:], AF.Copy)
                    for cq in range(4):
                        vq = pssm.tile([128, 128], BF16, tag="sm")
                        nc.tensor.transpose(vq[:], vt[:, cq * 128:(cq + 1) * 128],
                                            idn_sb[:])
                        ti = sc * 4 + cq
                        nc.vector.tensor_copy(
                            v_buf[:, ti * 128:(ti + 1) * 128], vq[:])

                    if sc == NJ - 1:
                        # prewarm the exp table; reading the last sdn pins the
                        # table swap after this batch's final Sqrt
                        warm = wrk.tile([128, 1], BF16, tag="warm", name="warm")
                        nc.scalar.activation(warm[:], sdn[:, 0:1], AF.Exp)
                        # zero fence carrying a data dep on the final Sqrt:
                        # biasing the first attention exps with it keeps the
                        # scheduler from interleaving them with P-phase Sqrts
                        # (each interleave costs a 1.3us activation-table swap)
                        fence = wrk.tile([128, 1], F32, tag="fence",
                                         name="fence")
                        nc.vector.tensor_scalar_mul(fence[:], sdn[:, 0:1], 0.0)
                        fence_ref[0] = fence

                if b == 0:
                    # wo weights are first needed ~90us in; loading them here
                    # keeps them out of the startup HBM crunch
                    for h in range(HL):
                        nc.gpsimd.dma_start(out=wo_sb[h][:],
                                            in_=wo[h * 128:(h + 1) * 128, :])

                # ======== phase A: attention (+ interleaved wo) ========
                def attn_chunk(h, j):
                    nsk = 4 * j + 4
                    if (h * NJ + j) % 2:
                        avp = psa.tile([128, SQC], F32, tag="av", name="avp")
                    else:
                        avp = pssm.tile([128, SQC], F32, tag="sm", name="avp")
                    # softmax denominator: off-diagonal e blocks are summed
                    # pairwise on vector (cheap bf16 adds, no serial chain),
                    # halving the PE ones-matmul columns; diagonal blocks go
                    # straight to the PE
                    dnp = psr.tile([128, SQC], F32, tag="row")
                    e_prev = None
                    dn_work = []  # (operand, lo) for end-of-chunk PE matmuls
                    for i in range(nsk):
                        lo = max(0, (i - 4 * j) * 128)
                        scp = psb.tile([128, SQC], F32, tag="big")
                        nc.tensor.matmul(
                            scp[:, lo:], kt_buf[:, i * 128:(i + 1) * 128],
                            qt_buf[h][:, j * SQC + lo:(j + 1) * SQC],
                            start=True, stop=True)
                        e = epool.tile([128, SQC], BF16, tag="e")
                        fb = (fence_ref[0][:]
                              if (h == 0 and j <= 1 and fence_ref[0]
                                  is not None) else 0.0)
                        nc.scalar.activation(e[:, lo:], scp[:, lo:],
                                             AF.Exp, scale=SCALE, bias=fb)
                        if i >= 4 * j:
                            nc.vector.tensor_mul(
                                e[:, lo:lo + 128], e[:, lo:lo + 128],
                                msk_sb[:, 0:128])
                        st, sp = i == 0, i == nsk - 1
                        nc.tensor.matmul(avp[:, lo:],
                                         v_buf[:, i * 128:(i + 1) * 128],
                                         e[:, lo:], start=st, stop=sp)
                        if i < 4 * j:
                            if i % 2 == 0:
                                e_prev = e
                            else:
                                pair = prp.tile([128, SQC], BF16, tag="pair")
                                nc.vector.tensor_add(pair[:], e_prev[:], e[:])
                                if i % 4 == 1:
                                    p_prev = pair
                                else:
                                    quad = prp.tile([128, SQC], BF16,
                                                    tag="quad")
                                    nc.vector.tensor_add(quad[:], p_prev[:],
                                                         pair[:])
                                    dn_work.append((quad, 0))
                        else:
                            dn_work.append((e, lo))
                    # denominator ones-matmuls issued after the QK/AV stream:
                    # their vector pair-adds are long done, so the in-order PE
                    # queue never stalls on a cross-engine hop
                    for dn, (op_t, lo) in enumerate(dn_work):
                        nc.tensor.matmul(dnp[:, lo:], on128_sb[:],
                                         op_t[:, lo:], start=(dn == 0),
                                         stop=(dn == len(dn_work) - 1))
                    rec = wrk.tile([128, SQC], F32, tag="bcs")
                    nc.vector.reciprocal_approx_fast(rec[:], dnp[:])
                    avs = wrk.tile([128, SQC], BF16, tag="avs")
                    nc.vector.tensor_copy(avs[:], avp[:])
                    nc.vector.tensor_mul(
                        ao_buf[h][:, j * SQC:(j + 1) * SQC],
                        avs[:], rec[:])

                def wo_chunk(j):
                    for dt in range(KT):
                        op = psb.tile([128, SQC], F32, tag="big", name="wop")
                        for h in range(HL):
                            nc.tensor.matmul(
                                op[:], wo_sb[h][:, dt * 128:(dt + 1) * 128],
                                ao_buf[h][:, j * SQC:(j + 1) * SQC],
                                start=(h == 0), stop=(h == HL - 1))
                        oo = oop.tile([128, SQC], BF16, tag="oo")
                        if dt % 2 == 0:
                            nc.scalar.activation(oo[:], op[:], AF.Copy)
                        else:
                            nc.vector.tensor_copy(oo[:], op[:])
                        (nc.sync if dt % 2 == 0 else nc.gpsimd).dma_start(
                            out=out[dt * 128:(dt + 1) * 128,
                                    base + j * SQC:base + (j + 1) * SQC],
                            in_=oo[:])

                # h0 ascending (chunk-0 inputs are ready first, no drain
                # stall at the P->A boundary); h1 descending with wo
                # interleaved one chunk behind so PE fills exp-wait gaps
                for j in range(NJ):
                    attn_chunk(0, j)
                for j in reversed(range(NJ)):
                    attn_chunk(1, j)
                    if j + 1 < NJ:
                        wo_chunk(j + 1)
                wo_chunk(0)


                if debug_dumps and b == 0:
                    nc.sync.dma_start(out=dbg["qt0d"], in_=qt_buf[0][:])
                    nc.sync.dma_start(out=dbg["ktd"], in_=kt_buf[:])
                    nc.sync.dma_start(out=dbg["vbd"], in_=v_buf[:])
                    nc.sync.dma_start(out=dbg["ao0d"], in_=ao_buf[0][:])
    nc.compile()
    return nc


_NC_CACHE = None


def _get_nc():
    global _NC_CACHE
    if _NC_CACHE is None:
        _NC_CACHE = build()
    return _NC_CACHE


def _bf(a):
    return np.ascontiguousarray(a.astype(ml_dtypes.bfloat16))


def _tile_w(w):
    cols = w.shape[1]
    return _bf(w.reshape(KT, 128, cols).transpose(1, 0, 2).reshape(128, KT * cols))


def kernel(x, wq, wk, wv, wo, q_gamma, k_gamma, cos_cache, sin_cache):
    x = np.asarray(x, dtype=np.float32)
    wq = np.asarray(wq, dtype=np.float32)
    wk = np.asarray(wk, dtype=np.float32)
    wv = np.asarray(wv, dtype=np.float32)
    wo = np.asarray(wo, dtype=np.float32)
    q_gamma = np.asarray(q_gamma, dtype=np.float32)
    k_gamma = np.asarray(k_gamma, dtype=np.float32)
    cos_cache = np.asarray(cos_cache, dtype=np.float32)
    sin_cache = np.asarray(sin_cache, dtype=np.float32)

    xt = _bf(x.reshape(BS, DIM).T)
    cos_t = cos_cache[:S].T
    sin_t = sin_cache[:S].T
    cost = _bf(np.concatenate([cos_t, cos_t], axis=0))
    # rotate_half on-chip is an unsigned partition swap; the sign of the
    # first half (-t2 * sin) is folded into the sin table here
    sint = _bf(np.concatenate([-sin_t, sin_t], axis=0))
    gq = np.ascontiguousarray(q_gamma[:, None])
    gk = np.ascontiguousarray(k_gamma[:, None])
    p = np.arange(128)[:, None]
    c = np.arange(128)[None, :]
    msk = _bf((p <= c).astype(np.float32))
    on128 = _bf(np.ones((128, 128), np.float32))
    giq_m = _bf(np.repeat((1.0 / (q_gamma * q_gamma))[:, None], 128, axis=1))
    gik_m = _bf(np.repeat((1.0 / (k_gamma * k_gamma))[:, None], 128, axis=1))
    idn = _bf(np.eye(128, dtype=np.float32))

    in_maps = []
    for cid in range(NCORES):
        in_maps.append({
            "xt": xt,
            "wq": _tile_w(wq[:, cid * HL * D:(cid + 1) * HL * D]),
            "wk": _tile_w(wk[:, cid * D:(cid + 1) * D]),
            "wv": _tile_w(wv[:, cid * D:(cid + 1) * D]),
            "wo": _bf(wo[cid * HL * D:(cid + 1) * HL * D, :]),
            "gq": gq, "gk": gk, "cost": cost, "sint": sint,
            "msk": msk, "on128": on128, "idn": idn,
            "giq": giq_m, "gik": gik_m,
        })

    nc = _get_nc()
    trace = os.environ.get("KERNEL_TRACE") == "1"
    r = run_bass_kernel_spmd(nc, in_maps, core_ids=list(range(NCORES)),
                             trace=trace)
    if trace:
        kernel.last_exec_time_ns = r.exec_time_ns
        kernel.last_results = r
    acc = np.zeros((DIM, BS), np.float32)
    for cid in range(NCORES):
        acc += r.results[cid]["out"].astype(np.float32)
    return np.ascontiguousarray(
        acc.T.reshape(B, S, DIM).astype(np.float32))
